# revision 1
# baseline (speedup 1.0000x reference)
"""Chunked cross attention (RETRO-style) Trainium2 Bass kernel.

Data-parallel over batch: 8 batch elements -> 8 NeuronCores, no collectives.
Matmul operands in bf16 (fp32 PSUM accumulation, fp32 LN/softmax stats);
measured fp32 matmul streaming runs at 1/4 rate on TRN2, bf16 at full rate.
Self-contained: hardcodes all shapes from the problem spec.
"""

import sys

sys.path.insert(0, "/opt/trn_rl_repo")

import numpy as np

import concourse.bass as bass
import concourse.mybir as mybir
import concourse.tile as tile
from concourse import bacc
from concourse.bass_utils import run_bass_kernel_spmd
from concourse.masks import make_identity

F32 = mybir.dt.float32
BF = mybir.dt.bfloat16

# Problem constants
D = 1024          # d_model
SEQ = 2048
CL = 64           # chunk len
CHUNKS = 32
NB = 2            # neighbors
NL = 128          # neighbor len
KV = NB * NL      # 256 kv tokens per chunk
H = 16            # heads
DK = 64           # head dim
P = 128
KT = D // P       # 8 k-tiles over d_model
LN_EPS = 1e-5
SCALE = 1.0 / np.sqrt(DK)
N_CORES = 8


def _bcast_ap(ap_1d, parts):
    """[N] AP -> [parts, N] AP with partition step 0 (for DMA broadcast)."""
    return bass.AP(
        tensor=ap_1d.tensor,
        offset=ap_1d.offset,
        ap=[[0, parts]] + list(ap_1d.ap),
    )


def build_program(repeat=1):
    nc = bacc.Bacc(None, target_bir_lowering=False, debug=False,
                   num_devices=N_CORES)

    h = nc.declare_dram_parameter("h", [SEQ, D], F32, isOutput=False)
    e = nc.declare_dram_parameter("e", [CHUNKS, NB, NL, D], F32, isOutput=False)
    Wq = nc.declare_dram_parameter("Wq", [D, D], F32, isOutput=False)
    bq = nc.declare_dram_parameter("bq", [D], F32, isOutput=False)
    Wk = nc.declare_dram_parameter("Wk", [D, D], F32, isOutput=False)
    bk = nc.declare_dram_parameter("bk", [D], F32, isOutput=False)
    Wv = nc.declare_dram_parameter("Wv", [D, D], F32, isOutput=False)
    bv = nc.declare_dram_parameter("bv", [D], F32, isOutput=False)
    gamma = nc.declare_dram_parameter("gamma", [D], F32, isOutput=False)
    beta = nc.declare_dram_parameter("beta", [D], F32, isOutput=False)
    Wo = nc.declare_dram_parameter("Wo", [D, D], F32, isOutput=False)
    bo = nc.declare_dram_parameter("bo", [D], F32, isOutput=False)
    out = nc.declare_dram_parameter("out", [SEQ, D], F32, isOutput=True)

    with tile.TileContext(nc) as tc:
        for _ in range(repeat):
            build_tile_kernel(nc, tc, h, e, Wq, bq, Wk, bk, Wv, bv, gamma,
                              beta, Wo, bo, out)
    nc.finalize()
    return nc


def build_tile_kernel(nc, tc, h, e, Wq, bq, Wk, bk, Wv, bv, gamma, beta,
                      Wo, bo, out):
    from contextlib import ExitStack

    ctx = ExitStack()
    with ctx:
        # ---------------- persistent pools ----------------
        singles = ctx.enter_context(tc.tile_pool(name="singles", bufs=1))
        wpool = ctx.enter_context(tc.tile_pool(name="weights", bufs=1))
        qtpool = ctx.enter_context(tc.tile_pool(name="qT_all", bufs=1))

        ident = singles.tile([P, P], BF)
        make_identity(nc, ident)

        eps_t = singles.tile([P, 1], F32)
        nc.vector.memset(eps_t, LN_EPS)

        # weights resident in SBUF as bf16 (cast during SWDGE DMA)
        def load_w(w, nm):
            t = wpool.tile([P, KT, D], BF, name=nm, tag=nm)
            nc.gpsimd.dma_start(
                out=t[:], in_=w[:].rearrange("(ko p) n -> p ko n", p=P))
            return t

        Wq_sb = load_w(Wq, "Wq_sb")
        Wk_sb = load_w(Wk, "Wk_sb")
        Wv_sb = load_w(Wv, "Wv_sb")
        Wo_sb = load_w(Wo, "Wo_sb")

        # biases (fp32): per-partition columns  bX_pm[p, m] = bX[m*128+p]
        bq_pm = singles.tile([P, KT], F32)
        nc.sync.dma_start(out=bq_pm[:], in_=bq[:].rearrange("(m p) -> p m", p=P))
        bk_pm = singles.tile([P, KT], F32)
        nc.sync.dma_start(out=bk_pm[:], in_=bk[:].rearrange("(m p) -> p m", p=P))

        # free-dim broadcasts (fp32)
        bv_b = singles.tile([P, D], F32)
        nc.gpsimd.dma_start(out=bv_b[:], in_=_bcast_ap(bv[:], P))
        bo_b = singles.tile([P, D], F32)
        nc.gpsimd.dma_start(out=bo_b[:], in_=_bcast_ap(bo[:], P))

        # qT kept fully in SBUF (bf16): [p, m, tok]
        qT_sb = qtpool.tile([P, KT, SEQ], BF)

        # prefix rows: out[0:63] = h[0:63]
        nc.sync.dma_start(out=out[0:CL - 1, :], in_=h[0:CL - 1, :])

        # ================= phase 1: LN + Q projection =================
        with ExitStack() as p1:
            p1s = p1.enter_context(tc.tile_pool(name="p1s", bufs=1))
            gamma_b = p1s.tile([P, D], F32)
            nc.gpsimd.dma_start(out=gamma_b[:], in_=_bcast_ap(gamma[:], P))
            beta_b = p1s.tile([P, D], F32)
            nc.gpsimd.dma_start(out=beta_b[:], in_=_bcast_ap(beta[:], P))

            xpool = p1.enter_context(tc.tile_pool(name="x", bufs=3))
            stat = p1.enter_context(tc.tile_pool(name="stat", bufs=4))
            xtp = p1.enter_context(tc.tile_pool(name="xT", bufs=2))
            ps_t = p1.enter_context(
                tc.tile_pool(name="ps_t1", bufs=2, space="PSUM"))
            ps_q = p1.enter_context(
                tc.tile_pool(name="ps_q1", bufs=2, space="PSUM"))

            GT = 256  # tokens per group
            for g in range(SEQ // GT):
                xT = xtp.tile([P, KT, GT], BF, tag="xT")
                for half in range(2):
                    tok0 = g * GT + half * P
                    r0 = tok0 + CL - 1
                    r1 = min(r0 + P, SEQ)
                    nrows = r1 - r0

                    x_t = xpool.tile([P, D], F32, tag="x_raw")
                    nc.sync.dma_start(out=x_t[:nrows], in_=h[r0:r1, :])

                    st = stat.tile([P, 2, 6], F32, tag="st")
                    nc.vector.bn_stats(out=st[:nrows, 0], in_=x_t[:nrows, 0:512])
                    nc.vector.bn_stats(out=st[:nrows, 1], in_=x_t[:nrows, 512:D])
                    mv = stat.tile([P, 2], F32, tag="mv")
                    nc.vector.bn_aggr(out=mv[:nrows], in_=st[:nrows])

                    rstd = stat.tile([P, 1], F32, tag="rstd")
                    nc.scalar.activation(out=rstd[:nrows], in_=mv[:nrows, 1:2],
                                         func=mybir.ActivationFunctionType.Sqrt,
                                         bias=eps_t[:nrows])
                    nc.vector.reciprocal(out=rstd[:nrows], in_=rstd[:nrows])
                    nmu = stat.tile([P, 1], F32, tag="nmu")
                    nc.vector.tensor_scalar_mul(nmu[:nrows], mv[:nrows, 0:1], -1.0)

                    xn = xpool.tile([P, D], F32, tag="x_n")
                    nc.vector.tensor_scalar(
                        xn[:nrows], x_t[:nrows], nmu[:nrows], rstd[:nrows],
                        mybir.AluOpType.add, mybir.AluOpType.mult)
                    nc.vector.tensor_mul(xn[:nrows], xn[:nrows], gamma_b[:nrows])
                    # final op casts to bf16; pad rows zeroed
                    xb = xpool.tile([P, D], BF, tag="x_b")
                    if nrows < P:
                        nc.vector.memset(xb, 0.0)
                    nc.vector.tensor_add(xb[:nrows], xn[:nrows], beta_b[:nrows])

                    for k in range(KT):
                        tp = ps_t.tile([P, P], BF, tag="tp", space="PSUM")
                        nc.tensor.transpose(tp[:], xb[:, k * P:(k + 1) * P],
                                            ident[:])
                        nc.scalar.copy(out=xT[:, k, half * P:(half + 1) * P],
                                       in_=tp[:])

                # Q projection for this group (N=256)
                for m in range(KT):
                    qp = ps_q.tile([P, GT], F32, tag="qp", space="PSUM")
                    for k in range(KT):
                        nc.tensor.matmul(qp[:], Wq_sb[:, k, m * P:(m + 1) * P],
                                         xT[:, k, :],
                                         start=(k == 0), stop=(k == KT - 1))
                    nc.vector.tensor_scalar_add(
                        qT_sb[:, m, g * GT:(g + 1) * GT], qp[:],
                        bq_pm[:, m:m + 1])

        # ================= phase 2: K/V + attention + out =================
        epool = ctx.enter_context(tc.tile_pool(name="e_nat", bufs=4))
        etp = ctx.enter_context(tc.tile_pool(name="eT", bufs=2))
        ktp = ctx.enter_context(tc.tile_pool(name="kT", bufs=2))
        vp = ctx.enter_context(tc.tile_pool(name="v", bufs=2))
        probsp = ctx.enter_context(tc.tile_pool(name="probs", bufs=9))
        ptp = ctx.enter_context(tc.tile_pool(name="pT", bufs=4))
        otp = ctx.enter_context(tc.tile_pool(name="oT", bufs=2))
        outp = ctx.enter_context(tc.tile_pool(name="out_sb", bufs=2))
        hrp = ctx.enter_context(tc.tile_pool(name="hres", bufs=2))
        smalls = ctx.enter_context(tc.tile_pool(name="smalls", bufs=8))

        ps_mm = ctx.enter_context(tc.tile_pool(name="ps_mm", bufs=2, space="PSUM"))
        ps_sc = ctx.enter_context(tc.tile_pool(name="ps_sc", bufs=2, space="PSUM"))
        ps_tr = ctx.enter_context(tc.tile_pool(name="ps_tr", bufs=2, space="PSUM"))
        ps_ot = ctx.enter_context(tc.tile_pool(name="ps_ot", bufs=2, space="PSUM"))

        # per-chunk state carried between pipeline stages
        state = {}

        def emit_proj(c):
            """load e (cast to bf16), transpose, kT & v projections."""
            e_nat = []
            for nb in range(NB):
                en = epool.tile([P, D], BF, tag="e_nat")
                nc.gpsimd.dma_start(out=en[:], in_=e[c, nb])
                e_nat.append(en)

            eT = etp.tile([P, KT, KV], BF, tag="eT")
            for k in range(KT):
                tp = ps_tr.tile([P, KV], BF, tag="tr", space="PSUM")
                for nb in range(NB):
                    nc.tensor.transpose(tp[:, nb * NL:(nb + 1) * NL],
                                        e_nat[nb][:, k * P:(k + 1) * P],
                                        ident[:])
                nc.scalar.copy(out=eT[:, k, :], in_=tp[:])

            kT_sb = ktp.tile([P, KT, KV], BF, tag="kT")
            sc_jobs = state.get("sc_jobs", [])
            for m in range(KT):
                kp = ps_mm.tile([P, 512], F32, tag="mm", space="PSUM")
                for k in range(KT):
                    nc.tensor.matmul(kp[:, :KV],
                                     Wk_sb[:, k, m * P:(m + 1) * P],
                                     eT[:, k, :],
                                     start=(k == 0), stop=(k == KT - 1))
                nc.vector.tensor_scalar_add(kT_sb[:, m, :], kp[:, :KV],
                                            bk_pm[:, m:m + 1])
                # interleave: scores of previous chunk, one head-pair per m
                if sc_jobs:
                    sc_jobs[m]()

            v_sb = vp.tile([P, NB, D], BF, tag="v")
            pv_jobs = state.get("pv_jobs", [])
            for mt in range(NB):
                for n2 in range(2):
                    vps = ps_mm.tile([P, 512], F32, tag="mm", space="PSUM")
                    for k in range(KT):
                        nc.tensor.matmul(vps[:],
                                         eT[:, k, mt * P:(mt + 1) * P],
                                         Wv_sb[:, k, n2 * 512:(n2 + 1) * 512],
                                         start=(k == 0), stop=(k == KT - 1))
                    nc.vector.tensor_add(v_sb[:, mt, n2 * 512:(n2 + 1) * 512],
                                         vps[:], bv_b[:, n2 * 512:(n2 + 1) * 512])
                    # interleave: probs-transpose + PV of previous chunk
                    if pv_jobs:
                        pv_jobs[2 * mt + n2]()

            state["cur"] = dict(kT=kT_sb, v=v_sb, c=c)

        def emit_attention(d):
            """queue attention jobs for chunk d (they run interleaved with
            the next chunk's projections)."""
            st = state["cur"]
            assert st["c"] == d
            kT_sb, v_sb = st["kT"], st["v"]

            parity = d % 2
            if parity == 0:
                state["ot"] = otp.tile([P, KT, 2 * CL], BF, tag="ot",
                                       name="ot_pair")
            ot = state["ot"]

            probs_t = [None] * KT

            def make_sc_job(hp):
                def job():
                    q_sl = qT_sb[:, hp, d * CL:(d + 1) * CL]
                    sc = ps_sc.tile([P, KV], F32, tag="sc", space="PSUM")
                    nc.tensor.matmul(sc[0:DK, :], q_sl[0:DK], kT_sb[0:DK, hp, :],
                                     start=True, stop=True,
                                     tile_position=(0, 0))
                    nc.tensor.matmul(sc[DK:P, :], q_sl[DK:P], kT_sb[DK:P, hp, :],
                                     start=True, stop=True,
                                     tile_position=(DK, DK))
                    pr = probsp.tile([P, KV], BF, tag="probs")
                    sm = smalls.tile([P, 1], F32, tag="sums")
                    nc.scalar.activation(out=pr[:], in_=sc[:],
                                         func=mybir.ActivationFunctionType.Exp,
                                         scale=float(SCALE), accum_out=sm[:])
                    rc = smalls.tile([P, 1], F32, tag="rec")
                    nc.vector.reciprocal(out=rc[:], in_=sm[:])
                    nc.vector.tensor_scalar_mul(pr[:], pr[:], rc[:])
                    probs_t[hp] = pr
                return job

            def make_pv_job(j):
                # j in 0..3 -> head pairs 2j, 2j+1
                def job():
                    otps = ps_ot.tile([P, P], F32, tag="ot", space="PSUM")
                    for u in range(2):
                        hp = 2 * j + u
                        pr = probs_t[hp]
                        # full 128x128 PE transposes: probs[:, kv-tile].T
                        # gives [kv, (qA|qB)] for both heads at once
                        ptps = ps_tr.tile([P, KV], BF, tag="tr", space="PSUM")
                        for kvt in range(2):
                            nc.tensor.transpose(
                                ptps[:, kvt * P:(kvt + 1) * P],
                                pr[:, kvt * P:(kvt + 1) * P], ident[:])
                        pt = ptp.tile([P, KV], BF, tag="pt")
                        nc.vector.tensor_copy(out=pt[:], in_=ptps[:])
                        for h2 in range(2):
                            head = 2 * hp + h2
                            for kvt in range(2):
                                nc.tensor.matmul(
                                    otps[h2 * DK:(h2 + 1) * DK,
                                         u * DK:(u + 1) * DK],
                                    v_sb[:, kvt, head * DK:(head + 1) * DK],
                                    pt[:, kvt * P + h2 * DK:
                                       kvt * P + (h2 + 1) * DK],
                                    start=(kvt == 0), stop=(kvt == 1),
                                    tile_position=(0, h2 * DK))
                    # copy both head-pairs' oT into the pair accumulator
                    for u in range(2):
                        hp = 2 * j + u
                        nc.scalar.copy(
                            out=ot[:, hp, parity * CL:(parity + 1) * CL],
                            in_=otps[:, u * DK:(u + 1) * DK])
                return job

            state["sc_jobs"] = [make_sc_job(hp) for hp in range(KT)]
            state["pv_jobs"] = [make_pv_job(j) for j in range(4)]

        def emit_oproj(d):
            """output projection + residual for the pair ending at chunk d."""
            pair = d // 2
            ot = state["ot"]
            r0 = CL - 1 + pair * P
            r1 = min(r0 + P, SEQ)
            nrows = r1 - r0

            hres = hrp.tile([P, D], F32, tag="hres")
            nc.sync.dma_start(out=hres[:nrows], in_=h[r0:r1, :])
            out_sb = outp.tile([P, D], F32, tag="out_sb")
            for n2 in range(2):
                ops = ps_mm.tile([P, 512], F32, tag="mm", space="PSUM")
                for k in range(KT):
                    nc.tensor.matmul(ops[:], ot[:, k, :],
                                     Wo_sb[:, k, n2 * 512:(n2 + 1) * 512],
                                     start=(k == 0), stop=(k == KT - 1))
                sl = slice(n2 * 512, (n2 + 1) * 512)
                nc.vector.tensor_add(out_sb[:, sl], ops[:], bo_b[:, sl])
                nc.vector.tensor_add(out_sb[:nrows, sl], out_sb[:nrows, sl],
                                     hres[:nrows, sl])
            nc.sync.dma_start(out=out[r0:r1, :], in_=out_sb[:nrows])

        # software pipeline: proj(c) runs the queued attention jobs of c-1;
        # oproj(c-1) is emitted before emit_attention(c) replaces state["ot"].
        for c in range(CHUNKS + 1):
            if c < CHUNKS:
                emit_proj(c)
            else:
                # drain final chunk's attention jobs
                for job in state["sc_jobs"]:
                    job()
                for job in state["pv_jobs"]:
                    job()
            if c >= 2 and c % 2 == 0:
                emit_oproj(c - 1)
            if c < CHUNKS:
                emit_attention(c)
        state.clear()


_CACHE = {}


def kernel(**inputs):
    inputs = {k: np.ascontiguousarray(np.asarray(v, dtype=np.float32))
              for k, v in inputs.items()}
    hB = inputs["h"]
    B = hB.shape[0]
    assert hB.shape == (B, SEQ, D)

    if "nc" not in _CACHE:
        _CACHE["nc"] = build_program()
    nc = _CACHE["nc"]

    names = ["h", "e", "Wq", "bq", "Wk", "bk", "Wv", "bv", "gamma", "beta",
             "Wo", "bo"]
    in_maps = []
    for b in range(B):
        m = {}
        for n in names:
            a = inputs[n]
            m[n] = a[b] if n in ("h", "e") else a
        in_maps.append(m)

    res = run_bass_kernel_spmd(nc, in_maps, core_ids=list(range(B)))
    return np.stack([res.results[b]["out"] for b in range(B)], axis=0)


if __name__ == "__main__":
    nc = build_program()
    print("built ok")



# revision 7
# speedup vs baseline: 3.5401x; 3.5401x over previous
"""Chunked cross attention (RETRO-style) Trainium2 Bass kernel.

Data-parallel over batch: 8 batch elements -> 8 NeuronCores, no collectives.

I/O strategy: the axon-tunneled PJRT path re-ships every declared input
per call and pays a per-parameter fixed cost, so all large inputs are
packed into ONE bf16 blob per core (h, e, Wq, Wk, Wv, Wo) plus one small
fp32 tensor for the six 1024-vectors (bq, bk, bv, bo, gamma, beta). The
output is bf16. This cuts per-call transfer ~2.6x vs shipping 12 fp32
tensors.

Device-side: matmul operands bf16 (fp32 PSUM accumulation, fp32
LN/softmax stats). e is transposed by the DMA xbar on load (no PE
transposes for eT). LayerNorm gamma is folded into Wq and beta into the
Q bias, removing two full-width DVE ops per token tile.

Self-contained: hardcodes all shapes from the problem spec.
"""

import sys

sys.path.insert(0, "/opt/trn_rl_repo")

import numpy as np
import ml_dtypes

import concourse.bass as bass
import concourse.mybir as mybir
import concourse.tile as tile
from concourse import bacc
from concourse.bass_utils import run_bass_kernel_spmd
from concourse.masks import make_identity

F32 = mybir.dt.float32
BF = mybir.dt.bfloat16
BF_NP = ml_dtypes.bfloat16

# Problem constants
D = 1024          # d_model
SEQ = 2048
CL = 64           # chunk len
CHUNKS = 32
NB = 2            # neighbors
NL = 128          # neighbor len
KV = NB * NL      # 256 kv tokens per chunk
H = 16            # heads
DK = 64           # head dim
P = 128
KT = D // P       # 8 k-tiles over d_model
LN_EPS = 1e-5
SCALE = 1.0 / np.sqrt(DK)
N_CORES = 8

# blob layout (element offsets, bf16)
SZ_H = SEQ * D                   # 2,097,152
SZ_E = CHUNKS * NB * NL * D      # 8,388,608
SZ_W = D * D                     # 1,048,576
OFF_H = 0
OFF_E = OFF_H + SZ_H
OFF_WQ = OFF_E + SZ_E
OFF_WK = OFF_WQ + SZ_W
OFF_WV = OFF_WK + SZ_W
OFF_WO = OFF_WV + SZ_W
SZ_BLOB = OFF_WO + SZ_W          # 14,680,064
E_ROWS = CHUNKS * NB * NL        # 8192


def _bcast_ap(ap_1d, parts):
    """[N] AP -> [parts, N] AP with partition step 0 (for DMA broadcast)."""
    return bass.AP(
        tensor=ap_1d.tensor,
        offset=ap_1d.offset,
        ap=[[0, parts]] + list(ap_1d.ap),
    )


def build_program(repeat=1):
    nc = bacc.Bacc(None, target_bir_lowering=False, debug=False,
                   num_devices=N_CORES)

    blob = nc.declare_dram_parameter("blob", [SZ_BLOB], BF, isOutput=False)
    vecs = nc.declare_dram_parameter("vecs", [6 * D], F32, isOutput=False)
    out = nc.declare_dram_parameter("out", [SEQ, D], BF, isOutput=True)

    with tile.TileContext(nc) as tc:
        for _ in range(repeat):
            build_tile_kernel(nc, tc, blob, vecs, out)
    nc.finalize()
    return nc


def build_tile_kernel(nc, tc, blob, vecs, out):
    from contextlib import ExitStack

    h_v = blob[OFF_H:OFF_H + SZ_H].rearrange("(s d) -> s d", d=D)
    e_v = blob[OFF_E:OFF_E + SZ_E].rearrange("(r d) -> r d", d=D)

    ctx = ExitStack()
    with ctx:
        # ---------------- persistent pools ----------------
        singles = ctx.enter_context(tc.tile_pool(name="singles", bufs=1))
        wpool = ctx.enter_context(tc.tile_pool(name="weights", bufs=1))
        qtpool = ctx.enter_context(tc.tile_pool(name="qT_all", bufs=1))

        ident = singles.tile([P, P], BF)
        make_identity(nc, ident)

        eps_t = singles.tile([P, 1], F32)
        nc.vector.memset(eps_t, LN_EPS)

        # weights resident in SBUF as bf16 (already bf16 in DRAM)
        def load_w(off, nm):
            t = wpool.tile([P, KT, D], BF, name=nm, tag=nm)
            nc.sync.dma_start(
                out=t[:],
                in_=blob[off:off + SZ_W].rearrange("(ko p n) -> p ko n",
                                                   p=P, n=D))
            return t

        Wq_sb = load_w(OFF_WQ, "Wq_sb")
        Wk_sb = load_w(OFF_WK, "Wk_sb")
        Wv_sb = load_w(OFF_WV, "Wv_sb")
        Wo_sb = load_w(OFF_WO, "Wo_sb")

        # per-partition column views  vX_pm[p, m] = vX[m*128+p]
        def load_pm(idx, dt=F32):
            t = singles.tile([P, KT], dt, name=f"pm{idx}", tag=f"pm{idx}")
            nc.sync.dma_start(
                out=t[:],
                in_=vecs[idx * D:(idx + 1) * D].rearrange("(m p) -> p m", p=P))
            return t

        bq_pm = load_pm(0)
        bk_pm = load_pm(1)
        gamma_pm = load_pm(4)
        beta_pm = load_pm(5)
        beta_bf = singles.tile([P, KT], BF)
        nc.vector.tensor_copy(out=beta_bf[:], in_=beta_pm[:])

        # free-dim broadcasts (fp32)
        bv_b = singles.tile([P, D], F32)
        nc.gpsimd.dma_start(out=bv_b[:], in_=_bcast_ap(vecs[2 * D:3 * D], P))
        bo_b = singles.tile([P, D], F32)
        nc.gpsimd.dma_start(out=bo_b[:], in_=_bcast_ap(vecs[3 * D:4 * D], P))

        # fold LN beta into the Q bias: bq_eff[m] = bq[m] + sum_d beta_d Wq[d,m]
        bq_eff = singles.tile([P, KT], F32)
        with tc.tile_pool(name="ps_fold", bufs=2, space="PSUM") as ps_fold:
            for mt in range(KT):
                fp = ps_fold.tile([P, 1], F32, tag="fold", space="PSUM")
                for ko in range(KT):
                    nc.tensor.matmul(fp[:], Wq_sb[:, ko, mt * P:(mt + 1) * P],
                                     beta_bf[:, ko:ko + 1],
                                     start=(ko == 0), stop=(ko == KT - 1))
                nc.vector.tensor_add(bq_eff[:, mt:mt + 1], fp[:],
                                     bq_pm[:, mt:mt + 1])
        # fold LN gamma into Wq rows (d on partitions): Wq'[d,:] = gamma_d*Wq[d,:]
        for ko in range(KT):
            nc.vector.tensor_scalar_mul(Wq_sb[:, ko, :], Wq_sb[:, ko, :],
                                        gamma_pm[:, ko:ko + 1])

        # qT kept fully in SBUF (bf16): [p, m, tok]
        qT_sb = qtpool.tile([P, KT, SEQ], BF)

        # prefix rows: out[0:63] = h[0:63]
        nc.sync.dma_start(out=out[0:CL - 1, :], in_=h_v[0:CL - 1, :])

        # ================= phase 1: LN + Q projection =================
        with ExitStack() as p1:
            xpool = p1.enter_context(tc.tile_pool(name="x", bufs=3))
            stat = p1.enter_context(tc.tile_pool(name="stat", bufs=4))
            xtp = p1.enter_context(tc.tile_pool(name="xT", bufs=2))
            ps_t = p1.enter_context(
                tc.tile_pool(name="ps_t1", bufs=2, space="PSUM"))
            ps_q = p1.enter_context(
                tc.tile_pool(name="ps_q1", bufs=2, space="PSUM"))

            GT = 256  # tokens per group
            for g in range(SEQ // GT):
                xT = xtp.tile([P, KT, GT], BF, tag="xT")
                for half in range(2):
                    tok0 = g * GT + half * P
                    r0 = tok0 + CL - 1
                    r1 = min(r0 + P, SEQ)
                    nrows = r1 - r0

                    x_t = xpool.tile([P, D], BF, tag="x_raw")
                    nc.sync.dma_start(out=x_t[:nrows], in_=h_v[r0:r1, :])

                    st = stat.tile([P, 2, 6], F32, tag="st")
                    nc.vector.bn_stats(out=st[:nrows, 0], in_=x_t[:nrows, 0:512])
                    nc.vector.bn_stats(out=st[:nrows, 1], in_=x_t[:nrows, 512:D])
                    mv = stat.tile([P, 2], F32, tag="mv")
                    nc.vector.bn_aggr(out=mv[:nrows], in_=st[:nrows])

                    rstd = stat.tile([P, 1], F32, tag="rstd")
                    nc.scalar.activation(out=rstd[:nrows], in_=mv[:nrows, 1:2],
                                         func=mybir.ActivationFunctionType.Sqrt,
                                         bias=eps_t[:nrows])
                    nc.vector.reciprocal(out=rstd[:nrows], in_=rstd[:nrows])
                    nmu = stat.tile([P, 1], F32, tag="nmu")
                    nc.vector.tensor_scalar_mul(nmu[:nrows], mv[:nrows, 0:1],
                                                -1.0)

                    # xb = (x - mu) * rstd   (gamma/beta folded into Wq/bias)
                    xb = xpool.tile([P, D], BF, tag="x_b")
                    if nrows < P:
                        nc.vector.memset(xb, 0.0)
                    nc.vector.tensor_scalar(
                        xb[:nrows], x_t[:nrows], nmu[:nrows], rstd[:nrows],
                        mybir.AluOpType.add, mybir.AluOpType.mult)

                    for k in range(KT):
                        tp = ps_t.tile([P, P], BF, tag="tp", space="PSUM")
                        nc.tensor.transpose(tp[:], xb[:, k * P:(k + 1) * P],
                                            ident[:])
                        nc.scalar.copy(out=xT[:, k, half * P:(half + 1) * P],
                                       in_=tp[:])

                # Q projection for this group (N=256)
                for m in range(KT):
                    qp = ps_q.tile([P, GT], F32, tag="qp", space="PSUM")
                    for k in range(KT):
                        nc.tensor.matmul(qp[:], Wq_sb[:, k, m * P:(m + 1) * P],
                                         xT[:, k, :],
                                         start=(k == 0), stop=(k == KT - 1))
                    nc.vector.tensor_scalar_add(
                        qT_sb[:, m, g * GT:(g + 1) * GT], qp[:],
                        bq_eff[:, m:m + 1])

        # ================= phase 2: K/V + attention + out =================
        etp = ctx.enter_context(tc.tile_pool(name="eT", bufs=3))
        ktp = ctx.enter_context(tc.tile_pool(name="kT", bufs=2))
        vp = ctx.enter_context(tc.tile_pool(name="v", bufs=2))
        probsp = ctx.enter_context(tc.tile_pool(name="probs", bufs=9))
        ptp = ctx.enter_context(tc.tile_pool(name="pT", bufs=4))
        otp = ctx.enter_context(tc.tile_pool(name="oT", bufs=2))
        outp = ctx.enter_context(tc.tile_pool(name="out_sb", bufs=2))
        hrp = ctx.enter_context(tc.tile_pool(name="hres", bufs=2))
        smalls = ctx.enter_context(tc.tile_pool(name="smalls", bufs=8))

        ps_mm = ctx.enter_context(tc.tile_pool(name="ps_mm", bufs=2, space="PSUM"))
        ps_sc = ctx.enter_context(tc.tile_pool(name="ps_sc", bufs=2, space="PSUM"))
        ps_tr = ctx.enter_context(tc.tile_pool(name="ps_tr", bufs=2, space="PSUM"))
        ps_ot = ctx.enter_context(tc.tile_pool(name="ps_ot", bufs=2, space="PSUM"))

        # per-chunk state carried between pipeline stages
        state = {}

        def emit_proj(c):
            """eT via DMA-xbar transpose, then kT & v projections."""
            eT = etp.tile([P, KT, KV], BF, tag="eT")
            e_rows = e_v[c * KV:(c + 1) * KV, :]
            for k in range(KT):
                nc.sync.dma_start(out=eT[:, k, :],
                                  in_=e_rows[:, k * P:(k + 1) * P],
                                  transpose=True)

            kT_sb = ktp.tile([P, KT, KV], BF, tag="kT")
            sc_jobs = state.get("sc_jobs", [])
            for m in range(KT):
                kp = ps_mm.tile([P, 512], F32, tag="mm", space="PSUM")
                for k in range(KT):
                    nc.tensor.matmul(kp[:, :KV],
                                     Wk_sb[:, k, m * P:(m + 1) * P],
                                     eT[:, k, :],
                                     start=(k == 0), stop=(k == KT - 1))
                nc.vector.tensor_scalar_add(kT_sb[:, m, :], kp[:, :KV],
                                            bk_pm[:, m:m + 1])
                # interleave: scores of previous chunk, one head-pair per m
                if sc_jobs:
                    sc_jobs[m]()

            v_sb = vp.tile([P, NB, D], BF, tag="v")
            pv_jobs = state.get("pv_jobs", [])
            for mt in range(NB):
                for n2 in range(2):
                    vps = ps_mm.tile([P, 512], F32, tag="mm", space="PSUM")
                    for k in range(KT):
                        nc.tensor.matmul(vps[:],
                                         eT[:, k, mt * P:(mt + 1) * P],
                                         Wv_sb[:, k, n2 * 512:(n2 + 1) * 512],
                                         start=(k == 0), stop=(k == KT - 1))
                    nc.vector.tensor_add(v_sb[:, mt, n2 * 512:(n2 + 1) * 512],
                                         vps[:], bv_b[:, n2 * 512:(n2 + 1) * 512])
                    # interleave: probs-transpose + PV of previous chunk
                    if pv_jobs:
                        pv_jobs[2 * mt + n2]()

            state["cur"] = dict(kT=kT_sb, v=v_sb, c=c)

        def emit_attention(d):
            """queue attention jobs for chunk d (they run interleaved with
            the next chunk's projections)."""
            st = state["cur"]
            assert st["c"] == d
            kT_sb, v_sb = st["kT"], st["v"]

            parity = d % 2
            if parity == 0:
                state["ot"] = otp.tile([P, KT, 2 * CL], BF, tag="ot",
                                       name="ot_pair")
            ot = state["ot"]

            probs_t = [None] * KT

            def make_sc_job(hp):
                def job():
                    q_sl = qT_sb[:, hp, d * CL:(d + 1) * CL]
                    sc = ps_sc.tile([P, KV], F32, tag="sc", space="PSUM")
                    nc.tensor.matmul(sc[0:DK, :], q_sl[0:DK], kT_sb[0:DK, hp, :],
                                     start=True, stop=True,
                                     tile_position=(0, 0))
                    nc.tensor.matmul(sc[DK:P, :], q_sl[DK:P], kT_sb[DK:P, hp, :],
                                     start=True, stop=True,
                                     tile_position=(DK, DK))
                    pr = probsp.tile([P, KV], BF, tag="probs")
                    sm = smalls.tile([P, 1], F32, tag="sums")
                    nc.scalar.activation(out=pr[:], in_=sc[:],
                                         func=mybir.ActivationFunctionType.Exp,
                                         scale=float(SCALE), accum_out=sm[:])
                    rc = smalls.tile([P, 1], F32, tag="rec")
                    nc.vector.reciprocal(out=rc[:], in_=sm[:])
                    nc.vector.tensor_scalar_mul(pr[:], pr[:], rc[:])
                    probs_t[hp] = pr
                return job

            def make_pv_job(j):
                # j in 0..3 -> head pairs 2j, 2j+1
                def job():
                    otps = ps_ot.tile([P, P], F32, tag="ot", space="PSUM")
                    for u in range(2):
                        hp = 2 * j + u
                        pr = probs_t[hp]
                        # full 128x128 PE transposes: probs[:, kv-tile].T
                        # gives [kv, (qA|qB)] for both heads at once
                        ptps = ps_tr.tile([P, KV], BF, tag="tr", space="PSUM")
                        for kvt in range(2):
                            nc.tensor.transpose(
                                ptps[:, kvt * P:(kvt + 1) * P],
                                pr[:, kvt * P:(kvt + 1) * P], ident[:])
                        pt = ptp.tile([P, KV], BF, tag="pt")
                        nc.vector.tensor_copy(out=pt[:], in_=ptps[:])
                        for h2 in range(2):
                            head = 2 * hp + h2
                            for kvt in range(2):
                                nc.tensor.matmul(
                                    otps[h2 * DK:(h2 + 1) * DK,
                                         u * DK:(u + 1) * DK],
                                    v_sb[:, kvt, head * DK:(head + 1) * DK],
                                    pt[:, kvt * P + h2 * DK:
                                       kvt * P + (h2 + 1) * DK],
                                    start=(kvt == 0), stop=(kvt == 1),
                                    tile_position=(0, h2 * DK))
                    # copy both head-pairs' oT into the pair accumulator
                    for u in range(2):
                        hp = 2 * j + u
                        nc.scalar.copy(
                            out=ot[:, hp, parity * CL:(parity + 1) * CL],
                            in_=otps[:, u * DK:(u + 1) * DK])
                return job

            state["sc_jobs"] = [make_sc_job(hp) for hp in range(KT)]
            state["pv_jobs"] = [make_pv_job(j) for j in range(4)]

        def emit_oproj(d):
            """output projection + residual for the pair ending at chunk d."""
            pair = d // 2
            ot = state["ot"]
            r0 = CL - 1 + pair * P
            r1 = min(r0 + P, SEQ)
            nrows = r1 - r0

            hres = hrp.tile([P, D], BF, tag="hres")
            nc.sync.dma_start(out=hres[:nrows], in_=h_v[r0:r1, :])
            out_sb = outp.tile([P, D], BF, tag="out_sb")
            for n2 in range(2):
                ops = ps_mm.tile([P, 512], F32, tag="mm", space="PSUM")
                for k in range(KT):
                    nc.tensor.matmul(ops[:], ot[:, k, :],
                                     Wo_sb[:, k, n2 * 512:(n2 + 1) * 512],
                                     start=(k == 0), stop=(k == KT - 1))
                sl = slice(n2 * 512, (n2 + 1) * 512)
                nc.vector.tensor_add(ops[:], ops[:], bo_b[:, sl])
                nc.vector.tensor_add(out_sb[:nrows, sl], ops[:nrows],
                                     hres[:nrows, sl])
            nc.sync.dma_start(out=out[r0:r1, :], in_=out_sb[:nrows])

        # software pipeline: proj(c) runs the queued attention jobs of c-1;
        # oproj(c-1) is emitted before emit_attention(c) replaces state["ot"].
        for c in range(CHUNKS + 1):
            if c < CHUNKS:
                emit_proj(c)
            else:
                # drain final chunk's attention jobs
                for job in state["sc_jobs"]:
                    job()
                for job in state["pv_jobs"]:
                    job()
            if c >= 2 and c % 2 == 0:
                emit_oproj(c - 1)
            if c < CHUNKS:
                emit_attention(c)
        state.clear()


_CACHE = {}


def make_in_maps(inputs):
    """Pack fp32 inputs into per-core {blob: bf16, vecs: f32} maps."""
    h = np.asarray(inputs["h"], np.float32)
    e = np.asarray(inputs["e"], np.float32)
    B = h.shape[0]

    w_blob = np.empty(4 * SZ_W, dtype=BF_NP)
    for i, nm in enumerate(("Wq", "Wk", "Wv", "Wo")):
        w_blob[i * SZ_W:(i + 1) * SZ_W] = \
            np.asarray(inputs[nm], np.float32).reshape(-1).astype(BF_NP)

    vecs = np.stack([np.asarray(inputs[nm], np.float32)
                     for nm in ("bq", "bk", "bv", "bo", "gamma", "beta")])
    vecs = np.ascontiguousarray(vecs, np.float32)

    in_maps = []
    for b in range(B):
        blob = np.empty(SZ_BLOB, dtype=BF_NP)
        blob[OFF_H:OFF_H + SZ_H] = h[b].reshape(-1).astype(BF_NP)
        blob[OFF_E:OFF_E + SZ_E] = e[b].reshape(-1).astype(BF_NP)
        blob[OFF_WQ:] = w_blob
        in_maps.append({"blob": blob, "vecs": vecs})
    return in_maps


def kernel(**inputs):
    B = np.asarray(inputs["h"]).shape[0]
    assert np.asarray(inputs["h"]).shape == (B, SEQ, D)

    if "nc" not in _CACHE:
        _CACHE["nc"] = build_program()
    nc = _CACHE["nc"]

    in_maps = make_in_maps(inputs)
    res = run_bass_kernel_spmd(nc, in_maps, core_ids=list(range(B)))
    return np.stack([res.results[b]["out"].astype(np.float32)
                     for b in range(B)], axis=0)


if __name__ == "__main__":
    nc = build_program()
    print("built ok")
